# revision 1
# baseline (speedup 1.0000x reference)
"""Trainium2 Bass kernel for single-head dot-product self-attention.

  reference:  Q = x@Wq, K = x@Wk, V = x@Wv          (per batch element)
              out = softmax(Q K^T / sqrt(512)) @ V

Sharding: data-parallel over batch B=8 -> one batch element per NeuronCore.
Per core everything stays on-chip (SBUF/PSUM); matmuls run in float32r
(TF32-class rounding, ~1.5e-4 rel err per matmul, 4x faster than fp32).

Layout strategy per core (plan "E"):
  - x [2048,512] is DMA'd naturally then PE-transposed once into
    xT [d,s] (d on partitions) because every projection contracts over d.
  - QT/KT are produced directly transposed ([u, s]) so the scores matmul
    S[q, k] = QT[:,q].T @ KT[:,k] has q on PSUM partitions and k on the
    free axis -> softmax reductions are free-axis ops; the denominator
    falls out of the Exp activation's accum_out for free.
  - attn rows are PE-transposed (4 at a time into one PSUM bank) to form
    the stationary operand of the PV matmul; V is used in natural [s, u]
    layout as the moving operand.  out = (attn^T.T @ V) * (1/den).
"""

import sys

sys.path.insert(0, "/opt/trn_rl_repo")

import numpy as np

import concourse.bass as bass  # noqa: F401  (AP types come through tile/bacc)
import concourse.mybir as mybir
import concourse.tile as tile
from concourse import bacc
from concourse.bass_utils import run_bass_kernel_spmd
from concourse.masks import make_identity

f32 = mybir.dt.float32
f32r = mybir.dt.float32r

B, S, D, U = 8, 2048, 512, 512
P = 128                 # partitions
SC = S // P             # 16 s-chunks
DC = D // P             # 4 d-chunks
UC = U // P             # 4 u-chunks
NT = S // 512           # 4 512-wide moving tiles across s/k
SCALE = 1.0 / float(np.sqrt(U))
EXP = mybir.ActivationFunctionType.Exp


def build_nc(repeat: int = 1):
    nc = bacc.Bacc("TRN2", target_bir_lowering=False, debug=False)
    x_d = nc.dram_tensor("x", [S, D], f32, kind="ExternalInput")
    w_d = {
        "q": nc.dram_tensor("Wq", [D, U], f32, kind="ExternalInput"),
        "k": nc.dram_tensor("Wk", [D, U], f32, kind="ExternalInput"),
        "v": nc.dram_tensor("Wv", [D, U], f32, kind="ExternalInput"),
    }
    out_d = nc.dram_tensor("out", [S, U], f32, kind="ExternalOutput")

    with tile.TileContext(nc) as tc:
        with (
            tc.tile_pool(name="persist", bufs=1) as persist,
            tc.tile_pool(name="outsb", bufs=2) as out_pool,
            tc.tile_pool(name="ps_mm", bufs=3, space="PSUM") as ps_mm,
            tc.tile_pool(name="ps_t", bufs=2, space="PSUM") as ps_t_pool,
            tc.tile_pool(name="ps_o", bufs=2, space="PSUM") as ps_o_pool,
        ):
            ident = persist.tile([P, P], f32, tag="ident")
            make_identity(nc, ident[:])

            QT = [persist.tile([P, S], f32r, tag=f"QT{u}", name=f"QT{u}")
                  for u in range(UC)]
            KT = [persist.tile([P, S], f32r, tag=f"KT{u}", name=f"KT{u}")
                  for u in range(UC)]
            V = [persist.tile([P, U], f32r, tag=f"V{s}", name=f"V{s}")
                 for s in range(SC)]

            for _rep in range(repeat):
                # ---------------- phase 1+2: load, transpose x, project QKV
                with (
                    tc.tile_pool(name="wstage", bufs=2) as wstage,
                    tc.tile_pool(name="wr", bufs=1) as wr_pool,
                    tc.tile_pool(name="xstage", bufs=6) as xstage,
                    tc.tile_pool(name="xT", bufs=1) as xT_pool,
                ):
                    wr = {}
                    for wname, wd in w_d.items():
                        for d in range(DC):
                            stg = wstage.tile([P, U], f32, tag="wstg",
                                              name="wstg")
                            nc.sync.dma_start(stg[:], wd[d * P:(d + 1) * P, :])
                            wrt = wr_pool.tile([P, U], f32r,
                                               tag=f"w{wname}{d}",
                                               name=f"w{wname}{d}")
                            nc.scalar.copy(wrt[:], stg[:])
                            wr[wname, d] = wrt

                    xT = [xT_pool.tile([P, S], f32r, tag=f"xT{d}",
                                       name=f"xT{d}") for d in range(DC)]
                    for sg in range(SC // 4):
                        xs = []
                        for j in range(4):
                            s = sg * 4 + j
                            t = xstage.tile([P, D], f32, tag="xs", name="xs")
                            nc.sync.dma_start(t[:], x_d[s * P:(s + 1) * P, :])
                            xs.append(t)
                        for d in range(DC):
                            pst = ps_t_pool.tile([P, 512], f32, tag="t",
                                                 name="pst")
                            for j in range(4):
                                nc.tensor.transpose(
                                    pst[:, j * P:(j + 1) * P],
                                    xs[j][:, d * P:(d + 1) * P], ident[:])
                            nc.vector.tensor_copy(
                                xT[d][:, sg * 512:(sg + 1) * 512], pst[:])

                    # QT/KT: [u, s] = Wq/Wk chunk^T . xT   (contract d)
                    for wname, dstT in (("q", QT), ("k", KT)):
                        for u in range(UC):
                            for st in range(NT):
                                ps = ps_mm.tile([P, 512], f32, tag="mm",
                                                name="ps")
                                for d in range(DC):
                                    nc.tensor.matmul(
                                        ps[:],
                                        wr[wname, d][:, u * P:(u + 1) * P],
                                        xT[d][:, st * 512:(st + 1) * 512],
                                        start=(d == 0), stop=(d == DC - 1))
                                nc.scalar.copy(
                                    dstT[u][:, st * 512:(st + 1) * 512],
                                    ps[:])

                    # V: [s, u] = xT chunk^T . Wv  (contract d)
                    for s in range(SC):
                        ps = ps_mm.tile([P, 512], f32, tag="mm", name="ps")
                        for d in range(DC):
                            nc.tensor.matmul(
                                ps[:], xT[d][:, s * P:(s + 1) * P],
                                wr["v", d][:],
                                start=(d == 0), stop=(d == DC - 1))
                        nc.scalar.copy(V[s][:], ps[:])

                # ---------------- phase 3: attention, one 128-row q chunk
                with (
                    tc.tile_pool(name="attnp", bufs=2) as attn_pool,
                    tc.tile_pool(name="attnT", bufs=2) as attnT_pool,
                ):
                    for q in range(SC):
                        attn = attn_pool.tile([P, S], f32, tag="attn",
                                              name="attn")
                        dpart = attn_pool.tile([P, NT], f32, tag="dpart",
                                               name="dpart")
                        for kt in range(NT):
                            ps = ps_mm.tile([P, 512], f32, tag="mm",
                                            name="ps")
                            for u in range(UC):
                                nc.tensor.matmul(
                                    ps[:], QT[u][:, q * P:(q + 1) * P],
                                    KT[u][:, kt * 512:(kt + 1) * 512],
                                    start=(u == 0), stop=(u == UC - 1))
                            nc.scalar.activation(
                                attn[:, kt * 512:(kt + 1) * 512], ps[:],
                                EXP, scale=SCALE,
                                accum_out=dpart[:, kt:kt + 1])
                        den = attn_pool.tile([P, 1], f32, tag="den",
                                             name="den")
                        nc.vector.reduce_sum(den[:], dpart[:],
                                             axis=mybir.AxisListType.X)
                        recip = attn_pool.tile([P, 1], f32, tag="recip",
                                               name="recip")
                        nc.vector.reciprocal(recip[:], den[:])

                        ps_o = ps_o_pool.tile([P, U], f32, tag="o",
                                              name="ps_o")
                        for kb in range(NT):
                            ps_t = ps_t_pool.tile([P, 512], f32, tag="t",
                                                  name="ps_t")
                            for j in range(4):
                                k = kb * 4 + j
                                nc.tensor.transpose(
                                    ps_t[:, j * P:(j + 1) * P],
                                    attn[:, k * P:(k + 1) * P], ident[:])
                            aT = attnT_pool.tile([P, 512], f32r,
                                                 tag=f"aT{kb}",
                                                 name=f"aT{kb}")
                            nc.vector.tensor_copy(aT[:], ps_t[:])
                            for j in range(4):
                                k = kb * 4 + j
                                nc.tensor.matmul(
                                    ps_o[:], aT[:, j * P:(j + 1) * P],
                                    V[k][:],
                                    start=(k == 0), stop=(k == SC - 1))
                        outt = out_pool.tile([P, U], f32, tag="out",
                                             name="outt")
                        nc.vector.tensor_scalar_mul(outt[:], ps_o[:],
                                                    recip[:])
                        nc.sync.dma_start(out_d[q * P:(q + 1) * P, :],
                                          outt[:])

    nc.finalize()
    return nc


def kernel(x: np.ndarray, Wq: np.ndarray, Wk: np.ndarray,
           Wv: np.ndarray) -> np.ndarray:
    x = np.ascontiguousarray(x, dtype=np.float32)
    Wq = np.ascontiguousarray(Wq, dtype=np.float32)
    Wk = np.ascontiguousarray(Wk, dtype=np.float32)
    Wv = np.ascontiguousarray(Wv, dtype=np.float32)
    assert x.shape == (B, S, D)

    nc = build_nc()
    in_maps = [{"x": x[b], "Wq": Wq, "Wk": Wk, "Wv": Wv} for b in range(B)]
    res = run_bass_kernel_spmd(nc, in_maps, list(range(B)))
    return np.stack([res.results[b]["out"] for b in range(B)], axis=0)


if __name__ == "__main__":
    rng = np.random.default_rng(0)
    x = rng.standard_normal((B, S, D), dtype=np.float32)
    sc = 1.0 / np.sqrt(D)
    Wq = rng.standard_normal((D, U), dtype=np.float32) * sc
    Wk = rng.standard_normal((D, U), dtype=np.float32) * sc
    Wv = rng.standard_normal((D, U), dtype=np.float32) * sc
    out = kernel(x=x, Wq=Wq, Wk=Wk, Wv=Wv)
    print("out", out.shape, out.dtype)


# revision 2
# speedup vs baseline: 6.5834x; 6.5834x over previous
"""Trainium2 Bass kernel for single-head dot-product self-attention.

  reference:  Q = x@Wq, K = x@Wk, V = x@Wv          (per batch element)
              out = softmax(Q K^T / sqrt(512)) @ V

Sharding: data-parallel over batch B=8 -> one batch element per NeuronCore.
All matmuls run in float32r (TF32-class rounding, IEEE fp32 container,
~2.4e-4 rel err per rounding, ~4x faster than fp32 on the PE).

Layout strategy per core (transposed-scores / "plan D"):
  - x [2048,512] is DMA'd naturally then PE-transposed once into
    xT [d, s] (d on partitions): every projection contracts over d.
  - QT/KT are produced transposed ([u, s]); scores are computed
    TRANSPOSED: S^T[k, q] = KT_slice.T @ QT, k on PSUM partitions.
  - exp(S^T) tiles feed the PV matmul directly as the moving operand
    (stationary = natural-layout V slices), so no attention-matrix
    transposes are ever needed:  out^T[u, q] = sum_k V[k,u] expS^T[k,q].
  - softmax denominator: DVE accumulates expS^T tiles over k (free-axis
    q), then one ones-vector matmul reduces the 128 partitions, and a
    2KB DRAM round-trip redistributes den[1, 512] to per-partition
    [128, 4] for the final per-q scaling.
  - out^T is PE-transposed back to [q, u] (4x 128x128 per q-chunk into
    one PSUM bank), scaled by 1/den (per-partition scalar) and DMA'd out.
"""

import sys

sys.path.insert(0, "/opt/trn_rl_repo")

import numpy as np

import concourse.bass as bass  # noqa: F401
import concourse.mybir as mybir
import concourse.tile as tile
from concourse import bacc
from concourse.bass_utils import run_bass_kernel_spmd
from concourse.masks import make_identity

f32 = mybir.dt.float32
f32r = mybir.dt.float32r

B, S, D, U = 8, 2048, 512, 512
P = 128                 # partitions
SC = S // P             # 16 s-chunks (also k-chunks)
DC = D // P             # 4 d-chunks
UC = U // P             # 4 u-chunks
QT_TILES = S // 512     # 4 q-tiles of width 512
SCALE = 1.0 / float(np.sqrt(U))
EXP = mybir.ActivationFunctionType.Exp


def build_nc(repeat: int = 1):
    nc = bacc.Bacc("TRN2", target_bir_lowering=False, debug=False)
    x_d = nc.dram_tensor("x", [S, D], f32, kind="ExternalInput")
    w_d = {
        "q": nc.dram_tensor("Wq", [D, U], f32, kind="ExternalInput"),
        "k": nc.dram_tensor("Wk", [D, U], f32, kind="ExternalInput"),
        "v": nc.dram_tensor("Wv", [D, U], f32, kind="ExternalInput"),
    }
    out_d = nc.dram_tensor("out", [S, U], f32, kind="ExternalOutput")
    scratch_d = nc.dram_tensor("den_scratch", [QT_TILES, 512], f32)

    with tile.TileContext(nc) as tc:
        with (
            tc.tile_pool(name="persist", bufs=1) as persist,
            tc.tile_pool(name="outsb", bufs=2) as out_pool,
            tc.tile_pool(name="ps_mm", bufs=2, space="PSUM") as ps_mm,
            tc.tile_pool(name="ps_o", bufs=1, space="PSUM") as ps_o_pool,
            tc.tile_pool(name="ps_t", bufs=2, space="PSUM") as ps_t_pool,
        ):
            ident = persist.tile([P, P], f32, tag="ident")
            make_identity(nc, ident[:])
            ones_f = persist.tile([P, 1], f32, tag="ones_f")
            nc.gpsimd.memset(ones_f[:], 1.0)
            ones = persist.tile([P, 1], f32r, tag="ones")
            nc.vector.tensor_copy(ones[:], ones_f[:])

            QT = [persist.tile([P, S], f32r, tag=f"QT{u}", name=f"QT{u}")
                  for u in range(UC)]
            KT = [persist.tile([P, S], f32r, tag=f"KT{u}", name=f"KT{u}")
                  for u in range(UC)]
            V = [persist.tile([P, U], f32r, tag=f"V{s}", name=f"V{s}")
                 for s in range(SC)]

            for _rep in range(repeat):
                # ---------------- phase 1+2: load, transpose x, project QKV
                with (
                    tc.tile_pool(name="wstage", bufs=2) as wstage,
                    tc.tile_pool(name="wr", bufs=1) as wr_pool,
                    tc.tile_pool(name="xstage", bufs=6) as xstage,
                    tc.tile_pool(name="xT", bufs=1) as xT_pool,
                ):
                    wr = {}
                    for wname, wd in w_d.items():
                        for d in range(DC):
                            stg = wstage.tile([P, U], f32, tag="wstg",
                                              name="wstg")
                            nc.sync.dma_start(stg[:], wd[d * P:(d + 1) * P, :])
                            wrt = wr_pool.tile([P, U], f32r,
                                               tag=f"w{wname}{d}",
                                               name=f"w{wname}{d}")
                            nc.scalar.copy(wrt[:], stg[:])
                            wr[wname, d] = wrt

                    xT = [xT_pool.tile([P, S], f32r, tag=f"xT{d}",
                                       name=f"xT{d}") for d in range(DC)]
                    for sg in range(SC // 4):
                        xs = []
                        for j in range(4):
                            s = sg * 4 + j
                            t = xstage.tile([P, D], f32, tag="xs", name="xs")
                            nc.sync.dma_start(t[:], x_d[s * P:(s + 1) * P, :])
                            xs.append(t)
                        for d in range(DC):
                            pst = ps_t_pool.tile([P, 512], f32, tag="t",
                                                 name="pst")
                            for j in range(4):
                                nc.tensor.transpose(
                                    pst[:, j * P:(j + 1) * P],
                                    xs[j][:, d * P:(d + 1) * P], ident[:])
                            nc.vector.tensor_copy(
                                xT[d][:, sg * 512:(sg + 1) * 512], pst[:])

                    # QT/KT: [u, s] = W chunk^T . xT   (contract d)
                    for wname, dstT in (("q", QT), ("k", KT)):
                        for u in range(UC):
                            for st in range(QT_TILES):
                                ps = ps_mm.tile([P, 512], f32, tag="mm",
                                                name="ps")
                                for d in range(DC):
                                    nc.tensor.matmul(
                                        ps[:],
                                        wr[wname, d][:, u * P:(u + 1) * P],
                                        xT[d][:, st * 512:(st + 1) * 512],
                                        start=(d == 0), stop=(d == DC - 1))
                                nc.scalar.copy(
                                    dstT[u][:, st * 512:(st + 1) * 512],
                                    ps[:])

                    # V: [s, u] = xT chunk^T . Wv  (contract d)
                    for s in range(SC):
                        ps = ps_mm.tile([P, 512], f32, tag="mm", name="ps")
                        for d in range(DC):
                            nc.tensor.matmul(
                                ps[:], xT[d][:, s * P:(s + 1) * P],
                                wr["v", d][:],
                                start=(d == 0), stop=(d == DC - 1))
                        nc.scalar.copy(V[s][:], ps[:])

                # ---------------- phase 3: attention per 512-wide q tile
                with (
                    tc.tile_pool(name="expp", bufs=1) as exp_pool,
                    tc.tile_pool(name="attn_sb", bufs=2) as attn_pool,
                ):
                    for qt in range(QT_TILES):
                        qs = qt * 512
                        expS = []
                        den_acc = attn_pool.tile([P, 512], f32,
                                                 tag="den_acc",
                                                 name="den_acc")
                        for k in range(SC):
                            ps = ps_mm.tile([P, 512], f32, tag="mm",
                                            name="ps")
                            for u in range(UC):
                                nc.tensor.matmul(
                                    ps[:], KT[u][:, k * P:(k + 1) * P],
                                    QT[u][:, qs:qs + 512],
                                    start=(u == 0), stop=(u == UC - 1))
                            e = exp_pool.tile([P, 512], f32r, tag=f"e{k}",
                                              name=f"e{k}")
                            nc.scalar.activation(e[:], ps[:], EXP,
                                                 scale=SCALE)
                            expS.append(e)
                            if k == 0:
                                nc.vector.tensor_copy(den_acc[:],
                                                      e[:].bitcast(f32))
                            else:
                                nc.vector.tensor_add(den_acc[:], den_acc[:],
                                                     e[:].bitcast(f32))

                        # 128-partition reduction of den_acc via ones-matmul
                        den_accr = attn_pool.tile([P, 512], f32r,
                                                  tag="den_accr",
                                                  name="den_accr")
                        nc.vector.tensor_copy(den_accr[:], den_acc[:])
                        ps_den = ps_t_pool.tile([1, 512], f32, tag="t",
                                                name="ps_den")
                        nc.tensor.matmul(ps_den[:], ones[:], den_accr[:],
                                         start=True, stop=True)
                        den_sb = attn_pool.tile([1, 512], f32, tag="den_sb",
                                                name="den_sb")
                        nc.vector.tensor_copy(den_sb[:], ps_den[:])
                        # [1,512] -> [128,4] via 2KB DRAM round-trip
                        nc.sync.dma_start(scratch_d[qt, :], den_sb[:1, :])
                        denT = attn_pool.tile([P, QT_TILES], f32, tag="denT",
                                              name="denT")
                        nc.sync.dma_start(
                            denT[:],
                            scratch_d[qt, :].rearrange("(j p) -> p j", p=P))
                        recipT = attn_pool.tile([P, QT_TILES], f32,
                                                tag="recipT", name="recipT")
                        nc.vector.reciprocal(recipT[:], denT[:])

                        # PV: out^T[u, q] = sum_k V[k][:,u].T @ expS[k]
                        ps_o = [ps_o_pool.tile([P, 512], f32, tag=f"o{u}",
                                               name=f"o{u}")
                                for u in range(UC)]
                        for k in range(SC):
                            for u in range(UC):
                                nc.tensor.matmul(
                                    ps_o[u][:],
                                    V[k][:, u * P:(u + 1) * P],
                                    expS[k][:],
                                    start=(k == 0), stop=(k == SC - 1))
                        o_sb = []
                        for u in range(UC):
                            ot = attn_pool.tile([P, 512], f32, tag=f"ot{u}",
                                                name=f"ot{u}")
                            nc.scalar.copy(ot[:], ps_o[u][:])
                            o_sb.append(ot)

                        # transpose out^T -> out rows, scale by 1/den, store
                        for c in range(4):
                            ps_t = ps_t_pool.tile([P, 512], f32, tag="t",
                                                  name="ps_t")
                            for u in range(UC):
                                nc.tensor.transpose(
                                    ps_t[:, u * P:(u + 1) * P],
                                    o_sb[u][:, c * P:(c + 1) * P], ident[:])
                            outt = out_pool.tile([P, U], f32, tag="out",
                                                 name="outt")
                            nc.vector.tensor_scalar_mul(outt[:], ps_t[:],
                                                        recipT[:, c:c + 1])
                            q0 = qs + c * P
                            nc.sync.dma_start(out_d[q0:q0 + P, :], outt[:])

    nc.finalize()
    return nc


def kernel(x: np.ndarray, Wq: np.ndarray, Wk: np.ndarray,
           Wv: np.ndarray) -> np.ndarray:
    x = np.ascontiguousarray(x, dtype=np.float32)
    Wq = np.ascontiguousarray(Wq, dtype=np.float32)
    Wk = np.ascontiguousarray(Wk, dtype=np.float32)
    Wv = np.ascontiguousarray(Wv, dtype=np.float32)
    assert x.shape == (B, S, D)

    nc = build_nc()
    in_maps = [{"x": x[b], "Wq": Wq, "Wk": Wk, "Wv": Wv} for b in range(B)]
    res = run_bass_kernel_spmd(nc, in_maps, list(range(B)))
    return np.stack([res.results[b]["out"] for b in range(B)], axis=0)


if __name__ == "__main__":
    rng = np.random.default_rng(0)
    x = rng.standard_normal((B, S, D), dtype=np.float32)
    sc = 1.0 / np.sqrt(D)
    Wq = rng.standard_normal((D, U), dtype=np.float32) * sc
    Wk = rng.standard_normal((D, U), dtype=np.float32) * sc
    Wv = rng.standard_normal((D, U), dtype=np.float32) * sc
    out = kernel(x=x, Wq=Wq, Wk=Wk, Wv=Wv)
    print("out", out.shape, out.dtype)
